# revision 1
# baseline (speedup 1.0000x reference)
"""AriaGroupedGEMM (MoE grouped GEMM) on 8 TRN2 NeuronCores.

Problem: input [4096, 2048] f32, weight [8, 2048, 2048] f32,
tokens_per_expert [8] int32 (tokens pre-sorted by expert).
out[i] = input[i] @ weight[expert_of(i)].

Strategy: expert-parallel. Core g owns expert g's weight and its token
group. Each core runs a dense [T_pad, 2048] @ [2048, 2048] GEMM in bf16
(fp32 PSUM accumulation). Host pre-swizzles operands into SBUF-native
layouts so every DMA is contiguous per partition line.

Schedule (trace-driven):
- The kernel head is input-bandwidth-bound: the PE can only work on what
  has arrived (~0.29 B/ns/core with all 8 cores streaming). To maximize
  work per fresh byte, phase A covers the 1024-column left half for ALL
  m-tiles at once (x-tiles amortize across the full 1024 width): 13.65us
  of matmuls per 4MB of data -> compute-bound, not supply-bound. Phase A
  uses all 8 PSUM banks (4 m-tiles x 2 banks). Phase B (right half) is
  pure streaming at the N=512 roofline.
- A (k-chunk, m) interleave in phase A consumes tiles in exact arrival
  order; DMAs are emitted in first-need order (per-ring FIFO => arrival
  order); the first items alternate both HWDGE rings, the bulk rides
  sync so the scalar engine stays free for casts + output DMA issues.
- Warm-up matmuls (own accumulation groups, on a phase-A psum bank)
  bridge engine-ready (~7.6us) to first-data (~11us) so the HAM clock
  gate is at 2.4GHz when real matmuls start and never re-throttles.
- PSUM->SBUF casts run on the Scalar engine; outputs are batched per
  (m, half) into [P,1024] staging tiles (few output DMAs); the final
  tile is computed as two 256-wide groups so only a small DMA trails
  the last matmul.
"""
import sys
import functools

for _p in ("/opt/trn_rl_repo", "/root/.axon_site/_ro/trn_rl_repo"):
    if _p not in sys.path:
        sys.path.insert(0, _p)

import numpy as np
import ml_dtypes

import concourse.mybir as mybir
import concourse.tile as tile
from concourse import bacc
from concourse import bass_utils

P = 128
K = 2048            # in_features (contraction)
N = 2048            # out_features
G = 8               # experts == cores
KO = K // P         # 16 k-subtiles
HALF = N // 2       # 1024: phase-A column width

COMPUTE_DT = mybir.dt.bfloat16
NP_COMPUTE = ml_dtypes.bfloat16
OUT_DT = mybir.dt.bfloat16      # psum(f32) -> bf16 on the way out; host upcasts

N_WARMUP_MM = 16    # N=256 warm-up matmuls sized to bridge engine-ready
                    # (~7.6us) to first-data (~11us) in either clock state

# DRAM w layout offsets (elements per partition line):
#   region A: [KO, 1024] k-major (cols 0:1024)          offset 0
#   region B3: [KO, 512] (cols 1024:1536)               offset KO*1024
#   region B4: [KO, 512] (cols 1536:2048)               offset KO*1536
OFF_A = 0
OFF_B3 = KO * HALF
OFF_B4 = KO * (HALF + 512)
WTOT = KO * N

@functools.lru_cache(maxsize=4)
def _build(t_pad: int):
    """Build + compile the per-core GEMM graph for token-pad t_pad."""
    mt = t_pad // P  # m tiles of 128 tokens

    nc = bacc.Bacc("TRN2", target_bir_lowering=False, debug=False)

    # xt[mi, p, ko, j] = X[mi*P + j, ko*P + p]
    xt_d = nc.dram_tensor(
        "xt", [mt, P, KO, P], COMPUTE_DT, kind="ExternalInput"
    ).ap()
    w_d = nc.dram_tensor("w", [P, WTOT], COMPUTE_DT, kind="ExternalInput").ap()
    out_d = nc.dram_tensor("out", [t_pad, N], OUT_DT, kind="ExternalOutput").ap()

    fast = mt <= 4  # phase-A needs 2 psum banks per m-tile

    with tile.TileContext(nc) as tc:
        with (
            tc.tile_pool(name="xt_p", bufs=1) as xt_p,
            tc.tile_pool(name="w_p", bufs=1) as w_p,
            tc.tile_pool(name="st_p", bufs=1) as st_p,
            tc.tile_pool(name="wu_p", bufs=1) as wu_p,
            tc.tile_pool(name="ps", bufs=8, space="PSUM") as ps,
        ):
            # phase-A psum tiles, allocated in the order their banks are
            # freed (casts fire per-m after its ko=15 matmuls) so phase-B's
            # pool cycling lines up with the frees
            psA = {}
            if fast:
                for m in range(mt):
                    for h in range(2):
                        psA[(m, h)] = ps.tile([P, 512], mybir.dt.float32,
                                              tag="psum", name=f"psA_{m}_{h}")

            # --- PE warm-up: independent single-MM groups. The wu tile is
            # memset on Vector (idle at kernel start); the target bank is a
            # phase-A psum tile (the first real group start=True clears it).
            wu = wu_p.tile([P, 256], COMPUTE_DT, tag="wu")
            nc.vector.memset(wu[:], 0.0)
            if fast:
                wu_ps = psA[(0, 0)]
            else:
                wu_ps = ps.tile([P, 256], mybir.dt.float32, tag="psum",
                                name="wu_ps")
            for i in range(N_WARMUP_MM):
                nc.tensor.matmul(wu_ps[:, 0:256], wu[:, 0:P], wu[:],
                                 start=True, stop=True, skip_group_check=True)

            # --- input DMAs in exact first-need order ---
            xt_t = {}           # (mi, quarter) -> (tile, base)
            wA_c = {}           # chunk -> tile [P, 2, 1024]
            wB_c = {}           # (b, c) -> tile [P, 8, 512]

            def dma_items():
                # wA chunk map: chunks 0,1 cover ko 0 and 1 ([P,1,1024],
                # small gating); chunks 2..8 cover ko-pairs ([P,2,1024])
                if fast:
                    # first-need order for ko-outer m-inner rounds:
                    # round ko needs xt[m] quarter ko//4 and wA ko-chunk
                    yield ("xt0", (0, 0))
                    yield ("wA", 0)
                    if mt > 1:
                        yield ("xth", (1, 0))
                    if mt > 2:
                        yield ("xth", (2, 0))
                    if mt > 3:
                        yield ("xth", (3, 0))
                    yield ("wA", 1)
                    yield ("wA", 2)
                    yield ("xt0", (0, 1))
                    yield ("wA", 3)
                    yield ("wA", 4)
                    yield ("xt0", (0, 2))
                    if mt > 1:
                        yield ("xth", (1, 1))
                    if mt > 2:
                        yield ("xth", (2, 1))
                    if mt > 3:
                        yield ("xth", (3, 1))
                    yield ("wA", 5)
                    yield ("wA", 6)
                    yield ("xt0", (0, 3))
                    yield ("wA", 7)
                    yield ("wA", 8)
                else:
                    for mi in range(mt):
                        yield ("xth", (mi, 0))
                        yield ("xth", (mi, 1))
                    for c in range(9):
                        yield ("wA", c)
                for b in (3, 4):
                    for c in range(2):
                        yield ("wB", (b, c))

            # head items alternate the two HWDGE rings (parallel first
            # arrivals); the bulk rides sync (FIFO preserves need order,
            # scalar engine stays free for casts + output DMAs)
            queues = [nc.sync, nc.scalar]
            qi = 0
            for idx, (kind, key) in enumerate(dma_items()):
                if idx < 8:
                    eng = queues[qi]
                    qi ^= 1
                else:
                    eng = nc.sync
                if kind == "xth":
                    mi, h = key
                    t = xt_p.tile([P, 8, P], COMPUTE_DT, tag=f"xt_m{mi}h{h}",
                                  name=f"xt_m{mi}h{h}")
                    eng.dma_start(t[:], xt_d[mi, :, h * 8:(h + 1) * 8, :])
                    xt_t[(mi, h * 2)] = (t, 0)
                    xt_t[(mi, h * 2 + 1)] = (t, 4)
                elif kind == "xt0":
                    mi, q4 = key
                    t = xt_p.tile([P, 4, P], COMPUTE_DT, tag=f"xt_m{mi}q{q4}",
                                  name=f"xt_m{mi}q{q4}")
                    eng.dma_start(t[:], xt_d[mi, :, q4 * 4:(q4 + 1) * 4, :])
                    xt_t[(mi, q4)] = (t, 0)
                elif kind == "wA":
                    c = key
                    nk = 1 if c < 2 else 2
                    ko0 = c if c < 2 else 2 * c - 2
                    t = w_p.tile([P, nk, HALF], COMPUTE_DT, tag=f"wA_{c}",
                                 name=f"wA_{c}")
                    o0 = OFF_A + ko0 * HALF
                    eng.dma_start(t[:], w_d[:, o0:o0 + nk * HALF])
                    wA_c[c] = t
                else:
                    b, c = key
                    off = OFF_B3 if b == 3 else OFF_B4
                    t = w_p.tile([P, 8, 512], COMPUTE_DT, tag=f"wB{b}_{c}",
                                 name=f"wB{b}_{c}")
                    o0 = off + c * 8 * 512
                    eng.dma_start(t[:], w_d[:, o0:o0 + 8 * 512])
                    wB_c[(b, c)] = t

            def xt_ap(mi, ko):
                t, base = xt_t[(mi, ko // 4)]
                return t[:, base + (ko % 4), :]

            def wA_ap(ko, h):
                if ko < 2:
                    c, j = ko, 0
                else:
                    c, j = (ko + 2) // 2, ko % 2
                return wA_c[c][:, j, h * 512:(h + 1) * 512]

            def wB_ap(b, ko, j0=0, w=512):
                return wB_c[(b, ko // 8)][:, ko % 8, j0:j0 + w]

            if fast:
                # output staging: per (m, half) [P, 1024] bf16; Scalar does
                # the casts AND the output DMA issues (same-engine FIFO)
                st = {(m, h): st_p.tile([P, HALF], OUT_DT, tag=f"st_{m}_{h}",
                                        name=f"st_{m}_{h}")
                      for m in range(mt) for h in range(2)}
                # --- phase A: left 1024 columns, ko-outer m-inner rounds.
                # Each round consumes only 256KB of fresh weight per 1.7us
                # of matmuls, so after the first rounds the phase is
                # compute-bound. Per-m casts fire right after that m's
                # ko=15 matmuls, overlapping the rest of the last round.
                for ko in range(KO):
                    for m in range(mt):
                        lhsT = xt_ap(m, ko)
                        for h in range(2):
                            nc.tensor.matmul(
                                psA[(m, h)][:], lhsT, wA_ap(ko, h),
                                start=(ko == 0), stop=(ko == KO - 1),
                            )
                        if ko == KO - 1:
                            nc.scalar.copy(st[(m, 0)][:, 0:512],
                                           psA[(m, 0)][:])
                            nc.scalar.copy(st[(m, 0)][:, 512:1024],
                                           psA[(m, 1)][:])
                            nc.scalar.dma_start(
                                out_d[m * P:(m + 1) * P, 0:HALF],
                                st[(m, 0)][:])

                # --- phase B: right 1024 columns, m-major per 512-block ---
                for b in (3, 4):
                    c0 = HALF if b == 3 else HALF + 512
                    so = 0 if b == 3 else 512
                    for m in range(mt):
                        last = b == 4 and m == mt - 1
                        if last:
                            # two 256-wide groups: group A's cast+DMA
                            # overlaps group B's matmuls -> short tail
                            nc.scalar.dma_start(
                                out_d[m * P:(m + 1) * P, HALF:HALF + 512],
                                st[(m, 1)][:, 0:512])
                            for g in range(2):
                                pg = ps.tile([P, 256], mybir.dt.float32,
                                             tag="psum", name=f"psum_l{g}")
                                for k in range(KO):
                                    nc.tensor.matmul(
                                        pg[:], xt_ap(m, k),
                                        wB_ap(b, k, g * 256, 256),
                                        start=(k == 0), stop=(k == KO - 1),
                                    )
                                if g == 0:
                                    nc.scalar.copy(
                                        st[(m, 1)][:, 512:768], pg[:])
                                else:
                                    # final cast on the idle Vector engine:
                                    # DVE copies 16-bit out at ~2x the ACT
                                    # rate, shortening the last-DMA chain
                                    nc.vector.tensor_copy(
                                        st[(m, 1)][:, 768:1024], pg[:])
                                eng = nc.scalar if g == 0 else nc.sync
                                eng.dma_start(
                                    out_d[m * P:(m + 1) * P,
                                          c0 + g * 256:c0 + (g + 1) * 256],
                                    st[(m, 1)][:, so + g * 256:
                                       so + (g + 1) * 256])
                            continue
                        psum_t = ps.tile([P, 512], mybir.dt.float32,
                                         tag="psum", name=f"psum_{b}_{m}")
                        for k in range(KO):
                            nc.tensor.matmul(
                                psum_t[:], xt_ap(m, k), wB_ap(b, k),
                                start=(k == 0), stop=(k == KO - 1),
                            )
                        nc.scalar.copy(st[(m, 1)][:, so:so + 512], psum_t[:])
                        if b == 4:
                            nc.scalar.dma_start(
                                out_d[m * P:(m + 1) * P, HALF:N],
                                st[(m, 1)][:])
            else:
                # generic fallback (mt > 4): m-major over four 512-blocks
                for bi in range(4):
                    for m in range(mt):
                        psum_t = ps.tile([P, 512], mybir.dt.float32,
                                         tag="psum", name=f"ps_{bi}_{m}")
                        for k in range(KO):
                            if bi < 2:
                                rhs = wA_ap(k, bi)
                            else:
                                rhs = wB_ap(bi + 1, k)
                            nc.tensor.matmul(
                                psum_t[:], xt_ap(m, k), rhs,
                                start=(k == 0), stop=(k == KO - 1),
                            )
                        o_sb = st_p.tile([P, 512], OUT_DT,
                                         tag=f"o{(bi * mt + m) % 4}",
                                         name=f"o_{bi}_{m}")
                        nc.scalar.copy(o_sb[:], psum_t[:])
                        nc.scalar.dma_start(
                            out_d[m * P:(m + 1) * P,
                                  bi * 512:(bi + 1) * 512], o_sb[:])

    nc.compile()
    return nc


def _swizzle_x(x_pad: np.ndarray, t_pad: int) -> np.ndarray:
    # [t_pad, K] f32 -> [mt, P, KO, P] bf16, xt[mi,p,ko,j] = X[mi*P+j, ko*P+p]
    mt = t_pad // P
    v = x_pad.reshape(mt, P, KO, P).transpose(0, 3, 2, 1)
    return np.ascontiguousarray(v.astype(NP_COMPUTE))


def _swizzle_w(w_g: np.ndarray) -> np.ndarray:
    # [K, N] f32 -> [P, WTOT]: region A = cols 0:1024 k-major,
    # region B3 = cols 1024:1536, B4 = cols 1536:2048 (k-major each);
    # every DMA chunk is one contiguous run per partition line
    v = w_g.reshape(KO, P, N).transpose(1, 0, 2)  # [P, KO, N]
    parts = [
        np.ascontiguousarray(v[:, :, 0:HALF]).reshape(P, KO * HALF),
        np.ascontiguousarray(v[:, :, HALF:HALF + 512]).reshape(P, KO * 512),
        np.ascontiguousarray(v[:, :, HALF + 512:N]).reshape(P, KO * 512),
    ]
    return np.ascontiguousarray(np.concatenate(parts, axis=1).astype(NP_COMPUTE))


def _run(input, weight, tokens_per_expert, trace=False, **trace_kwargs):
    inp = np.ascontiguousarray(np.asarray(input), dtype=np.float32)
    wgt = np.ascontiguousarray(np.asarray(weight), dtype=np.float32)
    counts = np.asarray(tokens_per_expert).astype(np.int64)
    num_tokens, k = inp.shape
    assert k == K and wgt.shape == (G, K, N)
    # token group boundaries (matches searchsorted(cumsum, arange, 'right')),
    # clamped to the token range for safety on degenerate counts
    ends = np.minimum(np.cumsum(counts), num_tokens)
    starts = np.minimum(ends - counts, num_tokens)
    sizes = np.maximum(ends - starts, 0)

    t_pad = max(P, int(-(-max(int(sizes.max()), 1) // P)) * P)
    nc = _build(t_pad)

    in_maps = []
    for g in range(G):
        x_pad = np.zeros((t_pad, K), dtype=np.float32)
        x_pad[: sizes[g]] = inp[starts[g]:ends[g]]
        in_maps.append({"xt": _swizzle_x(x_pad, t_pad), "w": _swizzle_w(wgt[g])})

    res = bass_utils.run_bass_kernel_spmd(
        nc, in_maps, core_ids=list(range(G)), trace=trace, **trace_kwargs
    )

    # tokens not covered by any expert group get zero output (matches the
    # reference's masked accumulation)
    out = np.zeros((num_tokens, N), dtype=np.float32)
    for g in range(G):
        out[starts[g]:ends[g]] = res.results[g]["out"][: sizes[g]].astype(np.float32)
    return out, res


def kernel(input, weight, tokens_per_expert):
    out, _ = _run(input, weight, tokens_per_expert)
    return out

